# revision 6
# baseline (speedup 1.0000x reference)
"""GuidedFilter (r=15, eps=0.5) Trainium2 Bass kernel, v4.

Full inputs: guide, input_map [16,1,1024,1024] f32. Data-parallel over 8
NeuronCores (2 images/core). Per image, per 128-row tile:
  - H direction (free axis): tensor_tensor_scan 31-tap window sums (DVE).
  - V direction (partition axis): PE band matmuls, bf16 weights, fp32 PSUM.
  - Elementwise chain in bf16 spread across DVE (2x tensor_tensor), Act
    (dtype converts, PSUM evacuation with fused 1/961 scale, exp/-ln
    reciprocal), and GPSIMD/Pool (bf16 multiplies, SBUF only).
Engine assignment chosen by balancing per-engine busy time.
"""

import numpy as np
import ml_dtypes

R = 15
K = 2 * R + 1  # 31
EPS = 0.5
NORM = 1.0 / (K * K)  # 1/961

_CACHE = {}


def _build_band_weights(Hc, NT):
    """Wf[k, m] = weight of input row k in output row m's reflect window."""
    Wf = np.zeros((Hc, Hc), np.float32)
    for m in range(Hc):
        for t in range(m - R, m + R + 1):
            k = t
            if k < 0:
                k = -k
            if k > Hc - 1:
                k = 2 * (Hc - 1) - k
            Wf[k, m] += 1.0
    # Pack per out-tile j into [128, 3*128]:
    #   cols 0:128   = center block  (in-tile j,   K=128)
    #   cols 128:256 = top edge      (in-tile j-1 rows 113:128, K=15, rows 64:128)
    #   cols 256:384 = bottom edge   (in-tile j+1 rows 0:15,    K=15, rows 0:15)
    wv = np.zeros((NT, 128, 384), np.float32)
    for j in range(NT):
        r0 = j * 128
        wv[j, :, 0:128] = Wf[r0 : r0 + 128, r0 : r0 + 128]
        if j > 0:
            wv[j, 64:128, 128:256] = Wf[r0 - 64 : r0, r0 : r0 + 128]
        if j < NT - 1:
            wv[j, 0:15, 256:384] = Wf[r0 + 128 : r0 + 143, r0 : r0 + 128]
    return wv.astype(ml_dtypes.bfloat16)


def build_nc(n_img, Hc, Wc):
    """Build the Bass module for one core processing n_img images of [Hc, Wc]."""
    import concourse.bass as bass
    import concourse.tile as tile
    from concourse import bacc, mybir

    P = 128
    NT = Hc // P
    PW = Wc + 32          # padded width; interior at cols 16..16+Wc
    CH = min(512, Wc)     # psum chunk width
    NC_ = Wc // CH        # chunks per tile
    f32 = mybir.dt.float32
    bf16 = mybir.dt.bfloat16
    AX = mybir.AxisListType.X
    OP = mybir.AluOpType
    AF = mybir.ActivationFunctionType

    nc = bacc.Bacc("TRN2", target_bir_lowering=False, debug=False)
    g_dram = nc.dram_tensor("guide", [n_img, Hc, Wc], f32, kind="ExternalInput")
    p_dram = nc.dram_tensor("input_map", [n_img, Hc, Wc], f32, kind="ExternalInput")
    wv_dram = nc.dram_tensor("wv", [NT, 128, 384], bf16, kind="ExternalInput")
    o_dram = nc.dram_tensor("out", [n_img, Hc, Wc], f32, kind="ExternalOutput")
    gap, pap, wap, oap = g_dram.ap(), p_dram.ap(), wv_dram.ap(), o_dram.ap()

    with tile.TileContext(nc) as tc:
        wpool = tc.alloc_tile_pool(name="wv", bufs=1)
        wv_sb = []
        for j in range(NT):
            wt = wpool.tile([128, 384], bf16, tag=f"wv{j}", name=f"wv{j}")
            nc.sync.dma_start(wt[:], wap[j])
            wv_sb.append(wt)

        raw_pool = tc.alloc_tile_pool(name="raw", bufs=2)      # xI/xP f32 raw
        i16_pool = tc.alloc_tile_pool(name="i16", bufs=5)      # I16 pad, image-long
        pad_pool = tc.alloc_tile_pool(name="pads", bufs=2)     # p16/Ip/II pads
        h_pool = tc.alloc_tile_pool(name="hx", bufs=5)         # 4 h tensors
        ab_pool = tc.alloc_tile_pool(name="ab", bufs=3)        # a/bb pads
        hab_pool = tc.alloc_tile_pool(name="hab", bufs=5)      # ha, hb
        ev_pool = tc.alloc_tile_pool(name="ev", bufs=2)        # A_* evacs
        cf_pool = tc.alloc_tile_pool(name="cf", bufs=2)        # chain transients
        o_pool = tc.alloc_tile_pool(name="o", bufs=3)
        ps_pool = tc.alloc_tile_pool(name="ps", bufs=1, space="PSUM")
        psab_pool = tc.alloc_tile_pool(name="psab", bufs=2, space="PSUM")

        def mirrors(xp):
            # left: cols 0:16 <- interior cols 32..17; right symmetric.
            c0 = 16 + Wc
            nc.vector.tensor_copy(out=xp[:, 0:16], in_=xp[:, 32:16:-1])
            nc.vector.tensor_copy(out=xp[:, c0 : c0 + 15], in_=xp[:, c0 - 2 : c0 - 17 : -1])

        def hscan(xp, out, dtag):
            init = cf_pool.tile([128, 1], f32, tag=f"init{dtag}", name=f"init{dtag}")
            nc.vector.reduce_sum(init[:], xp[:, 0:31], axis=AX)
            nc.vector.tensor_tensor_scan(
                out[:], xp[:, 31 : 31 + Wc], xp[:, 0:Wc], init[:],
                op0=OP.add, op1=OP.subtract,
            )

        def vpass(psum, hsrc, j, c):
            """psum[128, CH] = band-weighted column sums of hsrc tiles."""
            lo, hi = c * CH, (c + 1) * CH
            last_center = (j == 0 or hsrc[j - 1] is None) and (
                j == NT - 1 or hsrc[j + 1] is None
            )
            nc.tensor.matmul(
                psum[:], wv_sb[j][:, 0:128], hsrc[j][:, lo:hi],
                start=True, stop=last_center,
            )
            if j > 0 and hsrc[j - 1] is not None:
                nc.tensor.matmul(
                    psum[:], wv_sb[j][64:128, 128:256], hsrc[j - 1][64:128, lo:hi],
                    start=False, stop=(j == NT - 1 or hsrc[j + 1] is None),
                )
            if j < NT - 1 and hsrc[j + 1] is not None:
                nc.tensor.matmul(
                    psum[:], wv_sb[j][0:15, 256:384], hsrc[j + 1][0:15, lo:hi],
                    start=False, stop=True,
                )

        # per-image tile state; images are interleaved in the emission loop so
        # independent work from both images keeps every engine busy
        I16 = [[None] * NT for _ in range(n_img)]
        hI = [[None] * NT for _ in range(n_img)]
        hp = [[None] * NT for _ in range(n_img)]
        hIp = [[None] * NT for _ in range(n_img)]
        hII = [[None] * NT for _ in range(n_img)]
        ha = [[None] * NT for _ in range(n_img)]
        hb = [[None] * NT for _ in range(n_img)]

        if True:
            def stageAB(img, j):
                xI = raw_pool.tile([128, Wc], f32, tag="xI", name="xI")
                xP = raw_pool.tile([128, Wc], f32, tag="xP", name="xP")
                nc.sync.dma_start(xI[:], gap[img, j * 128 : (j + 1) * 128, :])
                nc.sync.dma_start(xP[:], pap[img, j * 128 : (j + 1) * 128, :])
                I16[img][j] = i16_pool.tile([128, PW], bf16, tag="I16", name="I16")
                p16 = pad_pool.tile([128, PW], bf16, tag="p16", name="p16")
                nc.scalar.copy(I16[img][j][:, 16 : 16 + Wc], xI[:])
                nc.scalar.copy(p16[:, 16 : 16 + Wc], xP[:])
                mirrors(I16[img][j])
                mirrors(p16)
                Ip16 = pad_pool.tile([128, PW], bf16, tag="Ip16", name="Ip16")
                II16 = pad_pool.tile([128, PW], bf16, tag="II16", name="II16")
                # full padded width products; pads of factors are mirrored so
                # the products are born padded (no extra mirror pass)
                nc.gpsimd.tensor_mul(Ip16[:], I16[img][j][:], p16[:])
                nc.gpsimd.tensor_mul(II16[:], I16[img][j][:], I16[img][j][:])
                hI[img][j] = h_pool.tile([128, Wc], bf16, tag="hI", name="hI")
                hp[img][j] = h_pool.tile([128, Wc], bf16, tag="hp", name="hp")
                hIp[img][j] = h_pool.tile([128, Wc], bf16, tag="hIp", name="hIp")
                hII[img][j] = h_pool.tile([128, Wc], bf16, tag="hII", name="hII")
                hscan(I16[img][j], hI[img][j], "I")
                hscan(p16, hp[img][j], "p")
                hscan(Ip16, hIp[img][j], "Ip")
                hscan(II16, hII[img][j], "II")

            def stageCD(img, j):
                A_I = ev_pool.tile([128, Wc], bf16, tag="A_I", name="A_I")
                A_p = ev_pool.tile([128, Wc], bf16, tag="A_p", name="A_p")
                A_Ip = ev_pool.tile([128, Wc], bf16, tag="A_Ip", name="A_Ip")
                A_IIe = ev_pool.tile([128, Wc], bf16, tag="A_IIe", name="A_IIe")
                for c in range(NC_):
                    lo, hi = c * CH, (c + 1) * CH
                    mI = ps_pool.tile([128, CH], f32, tag="psA", name="psA")
                    mp = ps_pool.tile([128, CH], f32, tag="psB", name="psB")
                    mIp = ps_pool.tile([128, CH], f32, tag="psC", name="psC")
                    mII = ps_pool.tile([128, CH], f32, tag="psD", name="psD")
                    vpass(mI, hI[img], j, c)
                    vpass(mp, hp[img], j, c)
                    vpass(mIp, hIp[img], j, c)
                    vpass(mII, hII[img], j, c)
                    nc.scalar.activation(A_I[:, lo:hi], mI[:], AF.Copy, scale=NORM)
                    nc.scalar.activation(A_p[:, lo:hi], mp[:], AF.Copy, scale=NORM)
                    nc.scalar.activation(A_Ip[:, lo:hi], mIp[:], AF.Copy, scale=NORM)
                    nc.scalar.activation(
                        A_IIe[:, lo:hi], mII[:], AF.Copy, scale=NORM, bias=EPS
                    )
                prod = cf_pool.tile([128, Wc], bf16, tag="prod", name="prod")
                nc.vector.tensor_mul(prod[:], A_I[:], A_p[:])
                cov = cf_pool.tile([128, Wc], bf16, tag="cov", name="cov")
                nc.vector.tensor_sub(cov[:], A_Ip[:], prod[:])
                sqI = cf_pool.tile([128, Wc], bf16, tag="sqI", name="sqI")
                nc.vector.tensor_mul(sqI[:], A_I[:], A_I[:])
                d2 = cf_pool.tile([128, Wc], bf16, tag="d2", name="d2")
                nc.vector.tensor_sub(d2[:], A_IIe[:], sqI[:])
                # r = 1/d2 via exp(-ln(d2)) on the Act engine
                lg = cf_pool.tile([128, Wc], bf16, tag="lg", name="lg")
                nc.scalar.activation(lg[:], d2[:], AF.Ln)
                r16 = cf_pool.tile([128, Wc], bf16, tag="r16", name="r16")
                nc.scalar.activation(r16[:], lg[:], AF.Exp, scale=-1.0)
                apad = ab_pool.tile([128, PW], bf16, tag="apad", name="apad")
                bpad = ab_pool.tile([128, PW], bf16, tag="bpad", name="bpad")
                av = apad[:, 16 : 16 + Wc]
                nc.vector.tensor_mul(av, cov[:], r16[:])
                t = cf_pool.tile([128, Wc], bf16, tag="t", name="t")
                nc.vector.tensor_mul(t[:], av, A_I[:])
                nc.gpsimd.tensor_sub(bpad[:, 16 : 16 + Wc], A_p[:], t[:])
                mirrors(apad)
                mirrors(bpad)
                ha[img][j] = hab_pool.tile([128, Wc], bf16, tag="ha", name="ha")
                hb[img][j] = hab_pool.tile([128, Wc], bf16, tag="hb", name="hb")
                hscan(apad, ha[img][j], "a")
                hscan(bpad, hb[img][j], "b")

            def stageF(img, j):
                Ma = cf_pool.tile([128, Wc], bf16, tag="Ma", name="Ma")
                Mb = cf_pool.tile([128, Wc], bf16, tag="Mb", name="Mb")
                for c in range(NC_):
                    lo, hi = c * CH, (c + 1) * CH
                    ma = psab_pool.tile([128, CH], f32, tag="psa", name="psa")
                    mb = psab_pool.tile([128, CH], f32, tag="psb", name="psb")
                    vpass(ma, ha[img], j, c)
                    vpass(mb, hb[img], j, c)
                    nc.scalar.activation(Ma[:, lo:hi], ma[:], AF.Copy, scale=NORM)
                    nc.scalar.activation(Mb[:, lo:hi], mb[:], AF.Copy, scale=NORM)
                o1 = cf_pool.tile([128, Wc], bf16, tag="o1", name="o1")
                nc.gpsimd.tensor_mul(o1[:], Ma[:], I16[img][j][:, 16 : 16 + Wc])
                o2 = o_pool.tile([128, Wc], f32, tag="o2", name="o2")
                nc.gpsimd.tensor_add(o2[:], o1[:], Mb[:])
                nc.sync.dma_start(
                    oap[img, j * 128 : (j + 1) * 128, :], o2[:]
                )

            # interleave images: global sequence of (img, j) tiles; AB leads
            # CD by LEAD slots, F lags CD by 1 slot
            seq = [(img, j) for j in range(NT) for img in range(n_img)]
            LEAD = 2
            n_slots = len(seq)
            for s in range(LEAD):
                stageAB(*seq[s])
            for s in range(n_slots):
                if s + LEAD < n_slots:
                    stageAB(*seq[s + LEAD])
                stageCD(*seq[s])
                if s >= 2:
                    stageF(*seq[s - 2])
            stageF(*seq[n_slots - 2])
            stageF(*seq[n_slots - 1])

        for _pool in (psab_pool, ps_pool, o_pool, cf_pool, ev_pool, hab_pool,
                      ab_pool, h_pool, pad_pool, i16_pool, raw_pool, wpool):
            _pool.release()

    nc.compile()
    return nc


def _get_nc(n_img, Hc, Wc):
    key = (n_img, Hc, Wc)
    if key not in _CACHE:
        _CACHE[key] = build_nc(n_img, Hc, Wc)
    return _CACHE[key]


def kernel(guide, input_map):
    from concourse.bass_utils import run_bass_kernel_spmd

    B, C, Hc, Wc = guide.shape
    n_cores = 8
    n_img = B // n_cores
    g = np.ascontiguousarray(guide.reshape(B, Hc, Wc), dtype=np.float32)
    p = np.ascontiguousarray(input_map.reshape(B, Hc, Wc), dtype=np.float32)
    wv = _build_band_weights(Hc, Hc // 128)
    nc = _get_nc(n_img, Hc, Wc)
    in_maps = [
        {
            "guide": g[i * n_img : (i + 1) * n_img],
            "input_map": p[i * n_img : (i + 1) * n_img],
            "wv": wv,
        }
        for i in range(n_cores)
    ]
    res = run_bass_kernel_spmd(nc, in_maps, core_ids=list(range(n_cores)))
    out = np.concatenate([res.results[i]["out"] for i in range(n_cores)], axis=0)
    return out.reshape(B, C, Hc, Wc).astype(np.float32)


# revision 7
# speedup vs baseline: 1.3624x; 1.3624x over previous
"""GuidedFilter (r=15, eps=0.5) Trainium2 Bass kernel, v5.

Full inputs: guide, input_map [16,1,1024,1024] f32. Data-parallel over 8
NeuronCores (2 images/core). Per image, per 128-row tile:
  - H direction (free axis): tensor_tensor_scan 31-tap window sums (DVE).
  - V direction (partition axis): PE band matmuls, bf16 weights, fp32 PSUM
    in [128,1024] 2-bank tiles so PSUM evacuation is one Act instr.
  - Elementwise chain in bf16 spread across DVE (2x tensor_tensor), Act
    (PSUM evacuation with fused 1/961 scale, Square, exp/-ln reciprocal),
    and GPSIMD/Pool (multiplies, SBUF only, dtype-blind cost).
Inputs stay f32 end-to-end where an engine's cost is dtype-blind (scans,
Pool products); bf16 only where DVE's 2x mode pays.
"""

import numpy as np
import ml_dtypes

R = 15
K = 2 * R + 1  # 31
EPS = 0.5
NORM = 1.0 / (K * K)  # 1/961

_CACHE = {}


def _build_band_weights(Hc, NT):
    """Wf[k, m] = weight of input row k in output row m's reflect window."""
    Wf = np.zeros((Hc, Hc), np.float32)
    for m in range(Hc):
        for t in range(m - R, m + R + 1):
            k = t
            if k < 0:
                k = -k
            if k > Hc - 1:
                k = 2 * (Hc - 1) - k
            Wf[k, m] += 1.0
    wv = np.zeros((NT, 128, 384), np.float32)
    for j in range(NT):
        r0 = j * 128
        wv[j, :, 0:128] = Wf[r0 : r0 + 128, r0 : r0 + 128]
        if j > 0:
            wv[j, 64:128, 128:256] = Wf[r0 - 64 : r0, r0 : r0 + 128]
        if j < NT - 1:
            wv[j, 0:15, 256:384] = Wf[r0 + 128 : r0 + 143, r0 : r0 + 128]
    return wv.astype(ml_dtypes.bfloat16)


def build_nc(n_img, Hc, Wc):
    """Build the Bass module for one core processing n_img images of [Hc, Wc]."""
    import concourse.bass as bass
    import concourse.tile as tile
    from concourse import bacc, mybir

    P = 128
    NT = Hc // P
    PW = Wc + 32          # padded width; interior at cols 16..16+Wc
    CH = min(512, Wc)     # matmul chunk width (one PSUM bank)
    NC_ = Wc // CH
    f32 = mybir.dt.float32
    bf16 = mybir.dt.bfloat16
    AX = mybir.AxisListType.X
    OP = mybir.AluOpType
    AF = mybir.ActivationFunctionType

    nc = bacc.Bacc("TRN2", target_bir_lowering=False, debug=False)
    g_dram = nc.dram_tensor("guide", [n_img, Hc, Wc], f32, kind="ExternalInput")
    p_dram = nc.dram_tensor("input_map", [n_img, Hc, Wc], f32, kind="ExternalInput")
    wv_dram = nc.dram_tensor("wv", [NT, 128, 384], bf16, kind="ExternalInput")
    o_dram = nc.dram_tensor("out", [n_img, Hc, Wc], f32, kind="ExternalOutput")
    gap, pap, wap, oap = g_dram.ap(), p_dram.ap(), wv_dram.ap(), o_dram.ap()

    with tile.TileContext(nc) as tc:
        wpool = tc.alloc_tile_pool(name="wv", bufs=1)
        wv_sb = []
        for j in range(NT):
            wt = wpool.tile([128, 384], bf16, tag=f"wv{j}", name=f"wv{j}")
            nc.sync.dma_start(wt[:], wap[j])
            wv_sb.append(wt)

        xi_pool = tc.alloc_tile_pool(name="xi", bufs=5)       # xI pad f32, image-long
        xp_pool = tc.alloc_tile_pool(name="xp", bufs=2)       # xP pad f32
        pad_pool = tc.alloc_tile_pool(name="pads", bufs=3)    # Ip/II bf16 pads
        h_pool = tc.alloc_tile_pool(name="hx", bufs=4)        # 4 h tensors
        ab_pool = tc.alloc_tile_pool(name="ab", bufs=3)       # a/bb pads
        hab_pool = tc.alloc_tile_pool(name="hab", bufs=4)     # ha, hb
        ev_pool = tc.alloc_tile_pool(name="ev", bufs=3)       # A_* evacs
        cf_pool = tc.alloc_tile_pool(name="cf", bufs=3)       # chain transients
        o_pool = tc.alloc_tile_pool(name="o", bufs=3)
        ps_pool = tc.alloc_tile_pool(name="ps", bufs=1, space="PSUM")
        psab_pool = tc.alloc_tile_pool(name="psab", bufs=1, space="PSUM")

        def mirrors(xp):
            c0 = 16 + Wc
            nc.vector.tensor_copy(out=xp[:, 0:16], in_=xp[:, 32:16:-1])
            nc.vector.tensor_copy(out=xp[:, c0 : c0 + 15], in_=xp[:, c0 - 2 : c0 - 17 : -1])

        def hscan(xp, out, dtag):
            init = cf_pool.tile([128, 1], f32, tag=f"init{dtag}", name=f"init{dtag}")
            nc.vector.reduce_sum(init[:], xp[:, 0:31], axis=AX)
            nc.vector.tensor_tensor_scan(
                out[:], xp[:, 31 : 31 + Wc], xp[:, 0:Wc], init[:],
                op0=OP.add, op1=OP.subtract,
            )

        def vpass(psum, hsrc, j):
            """psum[128, Wc] (2 banks) = band-weighted column sums of hsrc."""
            for c in range(NC_):
                lo, hi = c * CH, (c + 1) * CH
                last_center = (j == 0 or hsrc[j - 1] is None) and (
                    j == NT - 1 or hsrc[j + 1] is None
                )
                nc.tensor.matmul(
                    psum[:, lo:hi], wv_sb[j][:, 0:128], hsrc[j][:, lo:hi],
                    start=True, stop=last_center,
                )
                if j > 0 and hsrc[j - 1] is not None:
                    nc.tensor.matmul(
                        psum[:, lo:hi], wv_sb[j][64:128, 128:256],
                        hsrc[j - 1][64:128, lo:hi],
                        start=False, stop=(j == NT - 1 or hsrc[j + 1] is None),
                    )
                if j < NT - 1 and hsrc[j + 1] is not None:
                    nc.tensor.matmul(
                        psum[:, lo:hi], wv_sb[j][0:15, 256:384],
                        hsrc[j + 1][0:15, lo:hi],
                        start=False, stop=True,
                    )

        for img in range(n_img):
            xI = [None] * NT
            hI = [None] * NT
            hp = [None] * NT
            hIp = [None] * NT
            hII = [None] * NT
            ha = [None] * NT
            hb = [None] * NT

            def stageAB(j):
                xI[j] = xi_pool.tile([128, PW], f32, tag="xI", name="xI")
                xP = xp_pool.tile([128, PW], f32, tag="xP", name="xP")
                nc.sync.dma_start(xI[j][:, 16 : 16 + Wc], gap[img, j * 128 : (j + 1) * 128, :])
                nc.sync.dma_start(xP[:, 16 : 16 + Wc], pap[img, j * 128 : (j + 1) * 128, :])
                mirrors(xI[j])
                mirrors(xP)
                Ip16 = pad_pool.tile([128, PW], bf16, tag="Ip16", name="Ip16")
                II16 = pad_pool.tile([128, PW], bf16, tag="II16", name="II16")
                # Pool products are dtype-blind: read padded f32, write bf16
                nc.gpsimd.tensor_mul(Ip16[:], xI[j][:], xP[:])
                nc.gpsimd.tensor_mul(II16[:], xI[j][:], xI[j][:])
                hI[j] = h_pool.tile([128, Wc], bf16, tag="hI", name="hI")
                hp[j] = h_pool.tile([128, Wc], bf16, tag="hp", name="hp")
                hIp[j] = h_pool.tile([128, Wc], bf16, tag="hIp", name="hIp")
                hII[j] = h_pool.tile([128, Wc], bf16, tag="hII", name="hII")
                hscan(xI[j], hI[j], "I")
                hscan(xP, hp[j], "p")
                hscan(Ip16, hIp[j], "Ip")
                hscan(II16, hII[j], "II")

            def stageCD(j):
                A_I = ev_pool.tile([128, Wc], bf16, tag="A_I", name="A_I")
                A_p = ev_pool.tile([128, Wc], bf16, tag="A_p", name="A_p")
                A_Ip = ev_pool.tile([128, Wc], bf16, tag="A_Ip", name="A_Ip")
                A_IIe = ev_pool.tile([128, Wc], bf16, tag="A_IIe", name="A_IIe")
                psA = ps_pool.tile([128, Wc], f32, tag="psA", name="psA")
                psB = ps_pool.tile([128, Wc], f32, tag="psB", name="psB")
                vpass(psA, hI, j)
                vpass(psB, hp, j)
                nc.scalar.activation(A_I[:], psA[:], AF.Copy, scale=NORM)
                nc.scalar.activation(A_p[:], psB[:], AF.Copy, scale=NORM)
                psC = ps_pool.tile([128, Wc], f32, tag="psA", name="psC")
                psD = ps_pool.tile([128, Wc], f32, tag="psB", name="psD")
                vpass(psC, hIp, j)
                vpass(psD, hII, j)
                nc.scalar.activation(A_Ip[:], psC[:], AF.Copy, scale=NORM)
                nc.scalar.activation(A_IIe[:], psD[:], AF.Copy, scale=NORM, bias=EPS)
                prod = cf_pool.tile([128, Wc], bf16, tag="prod", name="prod")
                nc.vector.tensor_mul(prod[:], A_I[:], A_p[:])
                cov = cf_pool.tile([128, Wc], bf16, tag="cov", name="cov")
                nc.vector.tensor_sub(cov[:], A_Ip[:], prod[:])
                sqI = cf_pool.tile([128, Wc], bf16, tag="sqI", name="sqI")
                nc.scalar.activation(sqI[:], A_I[:], AF.Square)
                d2 = cf_pool.tile([128, Wc], bf16, tag="d2", name="d2")
                nc.vector.tensor_sub(d2[:], A_IIe[:], sqI[:])
                lg = cf_pool.tile([128, Wc], bf16, tag="lg", name="lg")
                nc.scalar.activation(lg[:], d2[:], AF.Ln)
                r16 = cf_pool.tile([128, Wc], bf16, tag="r16", name="r16")
                nc.scalar.activation(r16[:], lg[:], AF.Exp, scale=-1.0)
                apad = ab_pool.tile([128, PW], bf16, tag="apad", name="apad")
                bpad = ab_pool.tile([128, PW], bf16, tag="bpad", name="bpad")
                av = apad[:, 16 : 16 + Wc]
                nc.vector.tensor_mul(av, cov[:], r16[:])
                t = cf_pool.tile([128, Wc], bf16, tag="t", name="t")
                nc.vector.tensor_mul(t[:], av, A_I[:])
                nc.gpsimd.tensor_sub(bpad[:, 16 : 16 + Wc], A_p[:], t[:])
                mirrors(apad)
                mirrors(bpad)
                ha[j] = hab_pool.tile([128, Wc], bf16, tag="ha", name="ha")
                hb[j] = hab_pool.tile([128, Wc], bf16, tag="hb", name="hb")
                hscan(apad, ha[j], "a")
                hscan(bpad, hb[j], "b")

            def stageF(j):
                Ma = cf_pool.tile([128, Wc], bf16, tag="Ma", name="Ma")
                Mb = cf_pool.tile([128, Wc], bf16, tag="Mb", name="Mb")
                psa = psab_pool.tile([128, Wc], f32, tag="psa", name="psa")
                psb = psab_pool.tile([128, Wc], f32, tag="psb", name="psb")
                vpass(psa, ha, j)
                vpass(psb, hb, j)
                nc.scalar.activation(Ma[:], psa[:], AF.Copy, scale=NORM)
                nc.scalar.activation(Mb[:], psb[:], AF.Copy, scale=NORM)
                o1 = cf_pool.tile([128, Wc], bf16, tag="o1", name="o1")
                nc.gpsimd.tensor_mul(o1[:], Ma[:], xI[j][:, 16 : 16 + Wc])
                o2 = o_pool.tile([128, Wc], f32, tag="o2", name="o2")
                nc.gpsimd.tensor_add(o2[:], o1[:], Mb[:])
                nc.sync.dma_start(oap[img, j * 128 : (j + 1) * 128, :], o2[:])

            # software-pipelined emission: AB leads CD by 2 tiles, F lags CD by 1
            stageAB(0)
            if NT > 1:
                stageAB(1)
            for j in range(NT):
                if j + 2 < NT:
                    stageAB(j + 2)
                stageCD(j)
                if j >= 1:
                    stageF(j - 1)
            stageF(NT - 1)

        for _pool in (psab_pool, ps_pool, o_pool, cf_pool, ev_pool, hab_pool,
                      ab_pool, h_pool, pad_pool, xp_pool, xi_pool, wpool):
            _pool.release()

    nc.compile()
    return nc


def _get_nc(n_img, Hc, Wc):
    key = (n_img, Hc, Wc)
    if key not in _CACHE:
        _CACHE[key] = build_nc(n_img, Hc, Wc)
    return _CACHE[key]


def kernel(guide, input_map):
    from concourse.bass_utils import run_bass_kernel_spmd

    B, C, Hc, Wc = guide.shape
    n_cores = 8
    n_img = B // n_cores
    g = np.ascontiguousarray(guide.reshape(B, Hc, Wc), dtype=np.float32)
    p = np.ascontiguousarray(input_map.reshape(B, Hc, Wc), dtype=np.float32)
    wv = _build_band_weights(Hc, Hc // 128)
    nc = _get_nc(n_img, Hc, Wc)
    in_maps = [
        {
            "guide": g[i * n_img : (i + 1) * n_img],
            "input_map": p[i * n_img : (i + 1) * n_img],
            "wv": wv,
        }
        for i in range(n_cores)
    ]
    res = run_bass_kernel_spmd(nc, in_maps, core_ids=list(range(n_cores)))
    out = np.concatenate([res.results[i]["out"] for i in range(n_cores)], axis=0)
    return out.reshape(B, C, Hc, Wc).astype(np.float32)
